# revision 9
# baseline (speedup 1.0000x reference)
"""Causal self-attention (GQA + RoPE) on 8 Trainium2 NeuronCores.

Sharding: core c = (b, g) with b = c // 4 (batch), g = c % 4 (group of 4
consecutive Q heads; KV head g // 2). Each core computes the attention
output for its 4 heads and a partial out-projection through the matching
256-column slice of Wo. Host sums the 4 partials per batch and adds bo.

Per-core kernel (all activations kept in transposed [feature, T] layout):
  - q/k/v projections as fp32r matmuls contracting C on partitions
  - RoPE as x*cos + (R@x)*sin where R is a pair-rotation matrix (PE matmul)
  - scores computed pre-transposed sT[k, q] so softmax exp (ScalarE) lands
    directly in the layout the AV matmul needs; no PE transposes of exp
  - softmax denominator via a ones-column appended to V (M=65 AV matmul)
  - causal structure handled block-wise: full 128x512 blocks below the
    diagonal, masked 128x128 blocks on the diagonal
  - normalization (1/den) applied while evacuating the AV PSUM
"""

import sys

for _p in ("/opt/trn_rl_repo", "/opt/pypackages"):
    if _p not in sys.path:
        sys.path.append(_p)

from contextlib import ExitStack

import numpy as np

import concourse.bacc as bacc
import concourse.mybir as mybir
import concourse.tile as tile
from concourse.bass import ts
from concourse.bass_utils import run_bass_kernel_spmd

B, T, C = 2, 2048, 1024
HQ, HKV, HD = 16, 2, 64
F32 = mybir.dt.float32
F32R = mybir.dt.float32r
AF = mybir.ActivationFunctionType
NCC = C // 128  # 8 chunks of the contraction dim
NEG = -1.0e30
SCALE = 1.0 / 64.0  # the reference's double 1/sqrt(64) scaling





def _emit(nc, tc, ctx, d):
    sing = ctx.enter_context(tc.tile_pool(name="sing", bufs=1))

    xT_sb = sing.tile([128, NCC, T], F32R)
    wq_sb = sing.tile([128, NCC, 256], F32R)
    wkv_sb = sing.tile([128, NCC, 128], F32R)
    wo_sb = sing.tile([128, 2, C], F32R)
    bq_sb = sing.tile([128, 2], F32)
    bkv_sb = sing.tile([128, 1], F32)
    cos_sb = sing.tile([128, T], F32)
    sin_sb = sing.tile([128, T], F32)
    r2t_sb = sing.tile([128, 128], F32R)
    id_sb = sing.tile([128, 128], F32R)
    dm_sb = sing.tile([128, 128], F32)
    bsel_sb = sing.tile([1, 2, 128], F32R)
    qT_sb = sing.tile([128, 2, T], F32R)   # pair j: head 2j at parts 0:64, 2j+1 at 64:128
    kvT_sb = sing.tile([128, T], F32R)     # v at parts 0:64, k (pre-rope) at 64:128
    kT_sb = sing.tile([128, T], F32R)      # roped k, duplicated in both halves
    vA_sb = sing.tile([128, 16, 65], F32R)  # v[k-chunk, :64] + ones column
    yT_sb = sing.tile([128, 2, T], F32R)   # normalized attention out, pair layout

    # input DMAs
    for cc in range(NCC):
        nc.sync.dma_start(
            out=xT_sb[:, cc, :],
            in_=d["xT"].ap().rearrange("(cc p) t -> p cc t", p=128)[:, cc, :],
        )
    nc.sync.dma_start(out=wq_sb[:], in_=d["wq"].ap().rearrange("(cc p) m -> p cc m", p=128))
    nc.sync.dma_start(out=wkv_sb[:], in_=d["wkv"].ap().rearrange("(cc p) m -> p cc m", p=128))
    nc.sync.dma_start(out=wo_sb[:], in_=d["wo"].ap().rearrange("(j p) c -> p j c", p=128))
    nc.sync.dma_start(out=bq_sb[:], in_=d["bq"].ap())
    nc.sync.dma_start(out=bkv_sb[:], in_=d["bkv"].ap())
    nc.sync.dma_start(out=cos_sb[:], in_=d["cos2"].ap())
    nc.sync.dma_start(out=sin_sb[:], in_=d["sin2"].ap())
    nc.sync.dma_start(out=r2t_sb[:], in_=d["r2t"].ap())
    nc.sync.dma_start(out=id_sb[:], in_=d["ident"].ap())
    nc.sync.dma_start(out=dm_sb[:], in_=d["dmask"].ap())
    nc.sync.dma_start(out=bsel_sb[:], in_=d["bsel"].ap())

    # ---- phase 1: projections, RoPE, v transpose ----
    with tc.tile_pool(name="pp1", bufs=2, space="PSUM") as pp1, \
         tc.tile_pool(name="tmp1", bufs=2) as tmp1:
        # kv projection -> kvT_sb (v | k), with bias
        for ch in range(4):
            ps = pp1.tile([128, 512], F32, tag="proj")
            for cc in range(NCC):
                nc.tensor.matmul(
                    ps[:], wkv_sb[:, cc, :], xT_sb[:, cc, ts(ch, 512)],
                    start=(cc == 0), stop=(cc == NCC - 1),
                )
            nc.scalar.activation(
                out=kvT_sb[:, ts(ch, 512)], in_=ps[:],
                func=AF.Identity, bias=bkv_sb[:, 0:1], scale=1.0,
            )
        # v -> [Tk, 64] layout with ones column (for the denominator)
        for c16 in range(16):
            pv = pp1.tile([128, 64], F32R, tag="vt")
            nc.tensor.transpose(pv[:], kvT_sb[0:64, ts(c16, 128)], id_sb[0:64, 0:64])
            nc.vector.tensor_copy(vA_sb[:, c16, 0:64], pv[:])
        nc.vector.memset(vA_sb[:, :, 64:65].bitcast(F32), 1.0)
        # RoPE on k (lives at partitions 64:128); fp32r matmuls must write
        # PSUM at base partition 0, so the rotation lands at 0:64 and the
        # roped k is assembled at 0:64 then duplicated up to 64:128
        for ch in range(4):
            pr = pp1.tile([128, 512], F32, tag="rot")
            nc.tensor.matmul(
                pr[0:64, :], r2t_sb[64:128, 64:128],
                kvT_sb[64:128, ts(ch, 512)], start=True, stop=True,
            )
            t1 = tmp1.tile([128, 512], F32, tag="t1")
            t2 = tmp1.tile([128, 512], F32, tag="t2")
            nc.vector.tensor_mul(t1[0:64, :], kvT_sb[64:128, ts(ch, 512)], cos_sb[64:128, ts(ch, 512)])
            nc.vector.tensor_mul(t2[0:64, :], pr[0:64, :], sin_sb[0:64, ts(ch, 512)])
            nc.vector.tensor_add(kT_sb[0:64, ts(ch, 512)], t1[0:64, :], t2[0:64, :])
        nc.sync.dma_start(out=kT_sb[64:128, :], in_=kT_sb[0:64, :])
        # q projection + bias + RoPE (in pair layout)
        for j in range(2):
            for ch in range(4):
                ps = pp1.tile([128, 512], F32, tag="proj")
                for cc in range(NCC):
                    nc.tensor.matmul(
                        ps[:], wq_sb[:, cc, ts(j, 128)], xT_sb[:, cc, ts(ch, 512)],
                        start=(cc == 0), stop=(cc == NCC - 1),
                    )
                nc.scalar.activation(
                    out=qT_sb[:, j, ts(ch, 512)], in_=ps[:],
                    func=AF.Identity, bias=bq_sb[:, j:j + 1], scale=1.0,
                )
                pr = pp1.tile([128, 512], F32, tag="rot")
                nc.tensor.matmul(
                    pr[:], r2t_sb[:], qT_sb[:, j, ts(ch, 512)],
                    start=True, stop=True,
                )
                t1 = tmp1.tile([128, 512], F32, tag="t1")
                t2 = tmp1.tile([128, 512], F32, tag="t2")
                nc.vector.tensor_mul(t1[:], qT_sb[:, j, ts(ch, 512)], cos_sb[:, ts(ch, 512)])
                nc.vector.tensor_mul(t2[:], pr[:], sin_sb[:, ts(ch, 512)])
                nc.vector.tensor_add(qT_sb[:, j, ts(ch, 512)], t1[:], t2[:])

    # ---- phase 2: attention per head ----
    with tc.tile_pool(name="pps", bufs=3, space="PSUM") as pps, \
         tc.tile_pool(name="ppy", bufs=2, space="PSUM") as ppy, \
         tc.tile_pool(name="ppb", bufs=2, space="PSUM") as ppb, \
         tc.tile_pool(name="expp", bufs=3) as expp, \
         tc.tile_pool(name="nrm", bufs=2) as nrm:
        for h in range(4):
            j, b0 = h // 2, (h % 2) * 64
            for qb in range(4):
                py = ppy.tile([65, 512], F32, tag="y")
                for kb in range(4 * qb):
                    s_ = pps.tile([128, 512], F32, tag="s")
                    nc.tensor.matmul(
                        s_[:], kT_sb[b0:b0 + 64, ts(kb, 128)],
                        qT_sb[b0:b0 + 64, j, ts(qb, 512)], start=True, stop=True,
                    )
                    e_ = expp.tile([128, 512], F32R, tag="e")
                    nc.scalar.activation(out=e_[:], in_=s_[:], func=AF.Exp, scale=SCALE)
                    nc.tensor.matmul(
                        py[:], vA_sb[:, kb, :], e_[:],
                        start=(kb == 0), stop=False,
                    )
                # diagonal band: k-chunk 4qb+r covers q in [kb*128, (qb+1)*512);
                # only its first 128 columns straddle the diagonal and get masked
                for r in range(4):
                    kb = 4 * qb + r
                    width = 512 - 128 * r
                    qoff = kb * 128
                    sd = pps.tile([128, 512], F32, tag="s")
                    nc.tensor.matmul(
                        sd[:, 0:width], kT_sb[b0:b0 + 64, ts(kb, 128)],
                        qT_sb[b0:b0 + 64, j, qoff:qoff + width], start=True, stop=True,
                    )
                    nc.vector.tensor_add(sd[:, 0:128], sd[:, 0:128], dm_sb[:])
                    ed = expp.tile([128, 512], F32R, tag="e")
                    nc.scalar.activation(out=ed[:, 0:width], in_=sd[:, 0:width], func=AF.Exp, scale=SCALE)
                    # start/stop are bank-granular: start only on the very first
                    # matmul into py, stop only on the last
                    nc.tensor.matmul(
                        py[:, 128 * r:512], vA_sb[:, kb, :], ed[:, 0:width],
                        start=(qb == 0 and r == 0), stop=(r == 3),
                    )
                # denominator broadcast + normalize while evacuating PSUM
                dn = nrm.tile([1, 512], F32R, tag="dn")
                nc.vector.tensor_copy(dn[0:1, :], py[64:65, :])
                pb = ppb.tile([128, 512], F32, tag="b")
                nc.tensor.matmul(
                    pb[:], bsel_sb[0:1, h % 2, :], dn[0:1, :],
                    start=True, stop=True,
                )
                rd = nrm.tile([64, 512], F32, tag="rd")
                nc.vector.reciprocal(rd[:], pb[b0:b0 + 64, :])
                nc.vector.tensor_mul(
                    yT_sb[b0:b0 + 64, j, ts(qb, 512)], py[0:64, :], rd[:],
                )

    # ---- phase 3: out projection ----
    with tc.tile_pool(name="ppo", bufs=4, space="PSUM") as ppo, \
         tc.tile_pool(name="ost", bufs=3) as ost:
        for tq in range(16):
            for cf in range(2):
                po = ppo.tile([128, 512], F32, tag="o")
                for j in range(2):
                    nc.tensor.matmul(
                        po[:], yT_sb[:, j, ts(tq, 128)], wo_sb[:, j, ts(cf, 512)],
                        start=(j == 0), stop=(j == 1),
                    )
                ob = ost.tile([128, 512], F32, tag="ob")
                nc.vector.tensor_copy(ob[:], po[:])
                nc.sync.dma_start(out=d["out"].ap()[ts(tq, 128), ts(cf, 512)], in_=ob[:])


def build_program():
    nc = bacc.Bacc("TRN2", target_bir_lowering=False, debug=False, num_devices=8)
    d = {}
    MM_IN = {"xT", "wq", "wkv", "wo", "r2t", "ident", "bsel"}
    for name, shape in [
        ("xT", [C, T]), ("wq", [C, 256]), ("wkv", [C, 128]),
        ("bq", [128, 2]), ("bkv", [128, 1]), ("wo", [256, C]),
        ("cos2", [128, T]), ("sin2", [128, T]), ("r2t", [128, 128]),
        ("ident", [128, 128]), ("dmask", [128, 128]), ("bsel", [1, 2, 128]),
    ]:
        dt = F32R if name in MM_IN else F32
        d[name] = nc.dram_tensor(name, shape, dt, kind="ExternalInput")
    d["out"] = nc.dram_tensor("out", [T, C], F32, kind="ExternalOutput")
    with tile.TileContext(nc) as tc, ExitStack() as ctx:
        _emit(nc, tc, ctx, d)
    nc.compile()
    return nc


def host_prep(inputs):
    """Slice/transpose the full inputs into the 8 per-core input maps."""
    f = lambda a: np.ascontiguousarray(np.asarray(a, dtype=np.float32))
    x, rc = f(inputs["x"]), f(inputs["rope_cache"])
    Wq, bq = f(inputs["Wq"]), f(inputs["bq"])
    Wk, bk = f(inputs["Wk"]), f(inputs["bk"])
    Wv, bv = f(inputs["Wv"]), f(inputs["bv"])
    Wo = f(inputs["Wo"])

    cos2 = np.tile(np.repeat(rc[:, 1::2].T, 2, axis=0), (2, 1))  # [128, T]
    sin2 = np.tile(np.repeat(rc[:, 0::2].T, 2, axis=0), (2, 1))
    R2 = np.zeros((128, 128), np.float32)
    for i in range(64):
        R2[2 * i, 2 * i + 1] = -1.0
        R2[2 * i + 1, 2 * i] = 1.0
    r2t = np.ascontiguousarray(R2.T)
    ident = np.eye(128, dtype=np.float32)
    kk, qq = np.arange(128)[:, None], np.arange(128)[None, :]
    dmask = np.where(kk <= qq, 0.0, NEG).astype(np.float32)
    bsel = np.zeros((1, 2, 128), np.float32)
    bsel[0, 0, 0:64] = 1.0
    bsel[0, 1, 64:128] = 1.0

    in_maps = []
    for core in range(8):
        b, g = core // 4, core % 4
        kv = g // 2
        in_maps.append({
            "xT": np.ascontiguousarray(x[b].T),
            "wq": np.ascontiguousarray(Wq[256 * g:256 * (g + 1), :].T),
            "wkv": np.ascontiguousarray(np.concatenate(
                [Wv[64 * kv:64 * (kv + 1)].T, Wk[64 * kv:64 * (kv + 1)].T], axis=1)),
            "bq": np.ascontiguousarray(bq[256 * g:256 * (g + 1)].reshape(2, 128).T),
            "bkv": np.concatenate(
                [bv[64 * kv:64 * (kv + 1)], bk[64 * kv:64 * (kv + 1)]]).reshape(128, 1),
            "wo": np.ascontiguousarray(Wo[:, 256 * g:256 * (g + 1)].T),
            "cos2": cos2, "sin2": sin2, "r2t": r2t,
            "ident": ident, "dmask": dmask, "bsel": bsel,
        })
    return in_maps


_PROGRAM = None


def _get_program():
    global _PROGRAM
    if _PROGRAM is None:
        _PROGRAM = build_program()
    return _PROGRAM


def _gather(results, bo):
    full = np.empty((B, T, C), np.float32)
    for b in range(B):
        acc = results[4 * b]["out"].astype(np.float32).copy()
        for g in range(1, 4):
            acc += results[4 * b + g]["out"]
        full[b] = acc + bo
    return full


def kernel(**inputs):
    nc = _get_program()
    in_maps = host_prep(inputs)
    res = run_bass_kernel_spmd(nc, in_maps, list(range(8)))
    return _gather(res.results, np.asarray(inputs["bo"], np.float32))


def kernel_traced(**inputs):
    """Like kernel() but with NTFF tracing; returns (output, BassKernelResults)."""
    nc = _get_program()
    in_maps = host_prep(inputs)
    res = run_bass_kernel_spmd(nc, in_maps, list(range(8)), trace=True)
    return _gather(res.results, np.asarray(inputs["bo"], np.float32)), res


# revision 10
# speedup vs baseline: 1.3287x; 1.3287x over previous
"""Causal self-attention (GQA + RoPE) on 8 Trainium2 NeuronCores.

Sharding: core c = (b, g) with b = c // 4 (batch), g = c % 4 (group of 4
consecutive Q heads; KV head g // 2). Each core computes the attention
output for its 4 heads and a partial out-projection through the matching
256-column slice of Wo. Host sums the 4 partials per batch and adds bo.

Per-core kernel (all activations kept in transposed [feature, T] layout):
  - q/k/v projections as fp32r matmuls contracting C on partitions
  - RoPE as x*cos + (R@x)*sin where R is a pair-rotation matrix (PE matmul)
  - scores computed pre-transposed sT[k, q] so softmax exp (ScalarE) lands
    directly in the layout the AV matmul needs; no PE transposes of exp
  - softmax denominator via a ones-column appended to V (M=65 AV matmul)
  - causal structure handled block-wise: full 128x512 blocks below the
    diagonal, masked 128x128 blocks on the diagonal
  - normalization (1/den) applied while evacuating the AV PSUM
"""

import sys

for _p in ("/opt/trn_rl_repo", "/opt/pypackages"):
    if _p not in sys.path:
        sys.path.append(_p)

from contextlib import ExitStack

import numpy as np

import concourse.bacc as bacc
import concourse.mybir as mybir
import concourse.tile as tile
from concourse.bass import ts
from concourse.bass_utils import run_bass_kernel_spmd

B, T, C = 2, 2048, 1024
HQ, HKV, HD = 16, 2, 64
F32 = mybir.dt.float32
F32R = mybir.dt.float32r
AF = mybir.ActivationFunctionType
NCC = C // 128  # 8 chunks of the contraction dim
NEG = -1.0e30
SCALE = 1.0 / 64.0  # the reference's double 1/sqrt(64) scaling





def _emit(nc, tc, ctx, d):
    sing = ctx.enter_context(tc.tile_pool(name="sing", bufs=1))

    xT_sb = sing.tile([128, NCC, T], F32R)
    wq_sb = sing.tile([128, NCC, 256], F32R)
    wkv_sb = sing.tile([128, NCC, 128], F32R)
    wo_sb = sing.tile([128, 2, C], F32R)
    bq_sb = sing.tile([128, 2], F32)
    bkv_sb = sing.tile([128, 1], F32)
    cos_sb = sing.tile([128, T], F32)
    sin_sb = sing.tile([128, T], F32)
    r2t_sb = sing.tile([128, 128], F32R)
    id_sb = sing.tile([128, 128], F32R)
    dm_sb = sing.tile([128, 128], F32)
    bsel_sb = sing.tile([1, 2, 128], F32R)
    qT_sb = sing.tile([128, 2, T], F32R)   # pair j: head 2j at parts 0:64, 2j+1 at 64:128
    kvT_sb = sing.tile([128, T], F32R)     # v at parts 0:64, k (pre-rope) at 64:128
    kz0_sb = sing.tile([128, T], F32R)     # roped k at 0:64, zeros at 64:128
    kz1_sb = sing.tile([128, T], F32R)     # zeros at 0:64, roped k at 64:128
    vA_sb = sing.tile([128, 16, 128], F32R)  # v[k-chunk, :64] + ones col + zero pad
    yT_sb = sing.tile([128, 2, T], F32R)   # normalized attention out, pair layout

    # input DMAs
    for cc in range(NCC):
        nc.sync.dma_start(
            out=xT_sb[:, cc, :],
            in_=d["xT"].ap().rearrange("(cc p) t -> p cc t", p=128)[:, cc, :],
        )
    nc.sync.dma_start(out=wq_sb[:], in_=d["wq"].ap().rearrange("(cc p) m -> p cc m", p=128))
    nc.sync.dma_start(out=wkv_sb[:], in_=d["wkv"].ap().rearrange("(cc p) m -> p cc m", p=128))
    nc.sync.dma_start(out=wo_sb[:], in_=d["wo"].ap().rearrange("(j p) c -> p j c", p=128))
    nc.sync.dma_start(out=bq_sb[:], in_=d["bq"].ap())
    nc.sync.dma_start(out=bkv_sb[:], in_=d["bkv"].ap())
    nc.sync.dma_start(out=cos_sb[:], in_=d["cos2"].ap())
    nc.sync.dma_start(out=sin_sb[:], in_=d["sin2"].ap())
    nc.sync.dma_start(out=r2t_sb[:], in_=d["r2t"].ap())
    nc.sync.dma_start(out=id_sb[:], in_=d["ident"].ap())
    nc.sync.dma_start(out=dm_sb[:], in_=d["dmask"].ap())
    nc.sync.dma_start(out=bsel_sb[:], in_=d["bsel"].ap())

    # ---- phase 1: projections, RoPE, v transpose ----
    with tc.tile_pool(name="pp1", bufs=2, space="PSUM") as pp1, \
         tc.tile_pool(name="tmp1", bufs=2) as tmp1:
        # kv projection -> kvT_sb (v | k), with bias
        for ch in range(4):
            ps = pp1.tile([128, 512], F32, tag="proj")
            for cc in range(NCC):
                nc.tensor.matmul(
                    ps[:], wkv_sb[:, cc, :], xT_sb[:, cc, ts(ch, 512)],
                    start=(cc == 0), stop=(cc == NCC - 1),
                )
            nc.scalar.activation(
                out=kvT_sb[:, ts(ch, 512)], in_=ps[:],
                func=AF.Identity, bias=bkv_sb[:, 0:1], scale=1.0,
            )
        # v -> [Tk, 64] layout with ones column (for the denominator)
        for c16 in range(16):
            pv = pp1.tile([128, 64], F32R, tag="vt")
            nc.tensor.transpose(pv[:], kvT_sb[0:64, ts(c16, 128)], id_sb[0:64, 0:64])
            nc.vector.tensor_copy(vA_sb[:, c16, 0:64], pv[:])
        nc.vector.memset(vA_sb[:, :, 64:65].bitcast(F32), 1.0)
        nc.vector.memset(vA_sb[:, :, 65:128].bitcast(F32), 0.0)
        # RoPE on k (lives at partitions 64:128); fp32r matmuls must write
        # PSUM at base partition 0, so the rotation lands at 0:64 and the
        # roped k is assembled at 0:64 then duplicated up to 64:128
        for ch in range(4):
            pr = pp1.tile([128, 512], F32, tag="rot")
            nc.tensor.matmul(
                pr[0:64, :], r2t_sb[64:128, 64:128],
                kvT_sb[64:128, ts(ch, 512)], start=True, stop=True,
            )
            t1 = tmp1.tile([128, 512], F32, tag="t1")
            t2 = tmp1.tile([128, 512], F32, tag="t2")
            nc.vector.tensor_mul(t1[0:64, :], kvT_sb[64:128, ts(ch, 512)], cos_sb[64:128, ts(ch, 512)])
            nc.vector.tensor_mul(t2[0:64, :], pr[0:64, :], sin_sb[0:64, ts(ch, 512)])
            nc.vector.tensor_add(kz0_sb[0:64, ts(ch, 512)], t1[0:64, :], t2[0:64, :])
        nc.vector.memset(kz0_sb[64:128, :].bitcast(F32), 0.0)
        nc.vector.memset(kz1_sb[0:64, :].bitcast(F32), 0.0)
        nc.sync.dma_start(out=kz1_sb[64:128, :], in_=kz0_sb[0:64, :])
        # q projection + bias + RoPE (in pair layout)
        for j in range(2):
            for ch in range(4):
                ps = pp1.tile([128, 512], F32, tag="proj")
                for cc in range(NCC):
                    nc.tensor.matmul(
                        ps[:], wq_sb[:, cc, ts(j, 128)], xT_sb[:, cc, ts(ch, 512)],
                        start=(cc == 0), stop=(cc == NCC - 1),
                    )
                nc.scalar.activation(
                    out=qT_sb[:, j, ts(ch, 512)], in_=ps[:],
                    func=AF.Identity, bias=bq_sb[:, j:j + 1], scale=1.0,
                )
                pr = pp1.tile([128, 512], F32, tag="rot")
                nc.tensor.matmul(
                    pr[:], r2t_sb[:], qT_sb[:, j, ts(ch, 512)],
                    start=True, stop=True,
                )
                t1 = tmp1.tile([128, 512], F32, tag="t1")
                t2 = tmp1.tile([128, 512], F32, tag="t2")
                nc.vector.tensor_mul(t1[:], qT_sb[:, j, ts(ch, 512)], cos_sb[:, ts(ch, 512)])
                nc.vector.tensor_mul(t2[:], pr[:], sin_sb[:, ts(ch, 512)])
                nc.vector.tensor_add(qT_sb[:, j, ts(ch, 512)], t1[:], t2[:])

    # ---- phase 2: attention per head ----
    with tc.tile_pool(name="pps", bufs=3, space="PSUM") as pps, \
         tc.tile_pool(name="ppy", bufs=2, space="PSUM") as ppy, \
         tc.tile_pool(name="ppb", bufs=2, space="PSUM") as ppb, \
         tc.tile_pool(name="expp", bufs=3) as expp, \
         tc.tile_pool(name="nrm", bufs=2) as nrm:
        for h in range(4):
            j, b0 = h // 2, (h % 2) * 64
            for qb in range(4):
                py = ppy.tile([128, 512], F32, tag="y")
                kz = kz0_sb if h % 2 == 0 else kz1_sb
                for kb in range(4 * qb):
                    s_ = pps.tile([128, 512], F32, tag="s")
                    nc.tensor.matmul(
                        s_[:], kz[:, ts(kb, 128)],
                        qT_sb[:, j, ts(qb, 512)], start=True, stop=True,
                    )
                    e_ = expp.tile([128, 512], F32R, tag="e")
                    nc.scalar.activation(out=e_[:], in_=s_[:], func=AF.Exp, scale=SCALE)
                    nc.tensor.matmul(
                        py[:], vA_sb[:, kb, :], e_[:],
                        start=(kb == 0), stop=False,
                    )
                # diagonal band: k-chunk 4qb+r covers q in [kb*128, (qb+1)*512);
                # only its first 128 columns straddle the diagonal and get masked
                for r in range(4):
                    kb = 4 * qb + r
                    width = 512 - 128 * r
                    qoff = kb * 128
                    sd = pps.tile([128, 512], F32, tag="s")
                    nc.tensor.matmul(
                        sd[:, 0:width], kz[:, ts(kb, 128)],
                        qT_sb[:, j, qoff:qoff + width], start=True, stop=True,
                    )
                    nc.vector.tensor_add(sd[:, 0:128], sd[:, 0:128], dm_sb[:])
                    ed = expp.tile([128, 512], F32R, tag="e")
                    nc.scalar.activation(out=ed[:, 0:width], in_=sd[:, 0:width], func=AF.Exp, scale=SCALE)
                    # start/stop are bank-granular: start only on the very first
                    # matmul into py, stop only on the last
                    nc.tensor.matmul(
                        py[:, 128 * r:512], vA_sb[:, kb, :], ed[:, 0:width],
                        start=(qb == 0 and r == 0), stop=(r == 3),
                    )
                # denominator broadcast + normalize while evacuating PSUM
                dn = nrm.tile([1, 512], F32R, tag="dn")
                nc.vector.tensor_copy(dn[0:1, :], py[64:65, :])
                pb = ppb.tile([128, 512], F32, tag="b")
                nc.tensor.matmul(
                    pb[:], bsel_sb[0:1, h % 2, :], dn[0:1, :],
                    start=True, stop=True,
                )
                rd = nrm.tile([64, 512], F32, tag="rd")
                nc.vector.reciprocal(rd[:], pb[b0:b0 + 64, :])
                nc.vector.tensor_mul(
                    yT_sb[b0:b0 + 64, j, ts(qb, 512)], py[0:64, :], rd[:],
                )

    # ---- phase 3: out projection ----
    with tc.tile_pool(name="ppo", bufs=4, space="PSUM") as ppo, \
         tc.tile_pool(name="ost", bufs=3) as ost:
        for tq in range(16):
            for cf in range(2):
                po = ppo.tile([128, 512], F32, tag="o")
                for j in range(2):
                    nc.tensor.matmul(
                        po[:], yT_sb[:, j, ts(tq, 128)], wo_sb[:, j, ts(cf, 512)],
                        start=(j == 0), stop=(j == 1),
                    )
                ob = ost.tile([128, 512], F32, tag="ob")
                nc.vector.tensor_copy(ob[:], po[:])
                nc.sync.dma_start(out=d["out"].ap()[ts(tq, 128), ts(cf, 512)], in_=ob[:])


def build_program():
    nc = bacc.Bacc("TRN2", target_bir_lowering=False, debug=False, num_devices=8)
    d = {}
    MM_IN = {"xT", "wq", "wkv", "wo", "r2t", "ident", "bsel"}
    for name, shape in [
        ("xT", [C, T]), ("wq", [C, 256]), ("wkv", [C, 128]),
        ("bq", [128, 2]), ("bkv", [128, 1]), ("wo", [256, C]),
        ("cos2", [128, T]), ("sin2", [128, T]), ("r2t", [128, 128]),
        ("ident", [128, 128]), ("dmask", [128, 128]), ("bsel", [1, 2, 128]),
    ]:
        dt = F32R if name in MM_IN else F32
        d[name] = nc.dram_tensor(name, shape, dt, kind="ExternalInput")
    d["out"] = nc.dram_tensor("out", [T, C], F32, kind="ExternalOutput")
    with tile.TileContext(nc) as tc, ExitStack() as ctx:
        _emit(nc, tc, ctx, d)
    nc.compile()
    return nc


def host_prep(inputs):
    """Slice/transpose the full inputs into the 8 per-core input maps."""
    f = lambda a: np.ascontiguousarray(np.asarray(a, dtype=np.float32))
    x, rc = f(inputs["x"]), f(inputs["rope_cache"])
    Wq, bq = f(inputs["Wq"]), f(inputs["bq"])
    Wk, bk = f(inputs["Wk"]), f(inputs["bk"])
    Wv, bv = f(inputs["Wv"]), f(inputs["bv"])
    Wo = f(inputs["Wo"])

    cos2 = np.tile(np.repeat(rc[:, 1::2].T, 2, axis=0), (2, 1))  # [128, T]
    sin2 = np.tile(np.repeat(rc[:, 0::2].T, 2, axis=0), (2, 1))
    R2 = np.zeros((128, 128), np.float32)
    for i in range(64):
        R2[2 * i, 2 * i + 1] = -1.0
        R2[2 * i + 1, 2 * i] = 1.0
    r2t = np.ascontiguousarray(R2.T)
    ident = np.eye(128, dtype=np.float32)
    kk, qq = np.arange(128)[:, None], np.arange(128)[None, :]
    dmask = np.where(kk <= qq, 0.0, NEG).astype(np.float32)
    bsel = np.zeros((1, 2, 128), np.float32)
    bsel[0, 0, 0:64] = 1.0
    bsel[0, 1, 64:128] = 1.0

    in_maps = []
    for core in range(8):
        b, g = core // 4, core % 4
        kv = g // 2
        in_maps.append({
            "xT": np.ascontiguousarray(x[b].T),
            "wq": np.ascontiguousarray(Wq[256 * g:256 * (g + 1), :].T),
            "wkv": np.ascontiguousarray(np.concatenate(
                [Wv[64 * kv:64 * (kv + 1)].T, Wk[64 * kv:64 * (kv + 1)].T], axis=1)),
            "bq": np.ascontiguousarray(bq[256 * g:256 * (g + 1)].reshape(2, 128).T),
            "bkv": np.concatenate(
                [bv[64 * kv:64 * (kv + 1)], bk[64 * kv:64 * (kv + 1)]]).reshape(128, 1),
            "wo": np.ascontiguousarray(Wo[:, 256 * g:256 * (g + 1)].T),
            "cos2": cos2, "sin2": sin2, "r2t": r2t,
            "ident": ident, "dmask": dmask, "bsel": bsel,
        })
    return in_maps


_PROGRAM = None


def _get_program():
    global _PROGRAM
    if _PROGRAM is None:
        _PROGRAM = build_program()
    return _PROGRAM


def _gather(results, bo):
    full = np.empty((B, T, C), np.float32)
    for b in range(B):
        acc = results[4 * b]["out"].astype(np.float32).copy()
        for g in range(1, 4):
            acc += results[4 * b + g]["out"]
        full[b] = acc + bo
    return full


def kernel(**inputs):
    nc = _get_program()
    in_maps = host_prep(inputs)
    res = run_bass_kernel_spmd(nc, in_maps, list(range(8)))
    return _gather(res.results, np.asarray(inputs["bo"], np.float32))


def kernel_traced(**inputs):
    """Like kernel() but with NTFF tracing; returns (output, BassKernelResults)."""
    nc = _get_program()
    in_maps = host_prep(inputs)
    res = run_bass_kernel_spmd(nc, in_maps, list(range(8)), trace=True)
    return _gather(res.results, np.asarray(inputs["bo"], np.float32)), res


# revision 12
# speedup vs baseline: 1.5653x; 1.1780x over previous
"""Causal self-attention (GQA + RoPE) on 8 Trainium2 NeuronCores.

Sharding: core c = (b, g) with b = c // 4 (batch), g = c % 4 (group of 4
consecutive Q heads; KV head g // 2). Each core computes the attention
output for its 4 heads and a partial out-projection through the matching
256-column slice of Wo. Host sums the 4 partials per batch and adds bo.

Per-core kernel (all activations kept in transposed [feature, T] layout):
  - q/k/v projections as fp32r matmuls contracting C on partitions
  - RoPE as x*cos + (R@x)*sin where R is a pair-rotation matrix (PE matmul)
  - scores computed pre-transposed sT[k, q] so softmax exp (ScalarE) lands
    directly in the layout the AV matmul needs; no PE transposes of exp
  - softmax denominator via a ones-column appended to V (M=65 AV matmul)
  - causal structure handled block-wise: full 128x512 blocks below the
    diagonal, masked 128x128 blocks on the diagonal
  - normalization (1/den) applied while evacuating the AV PSUM
"""

import sys

for _p in ("/opt/trn_rl_repo", "/opt/pypackages"):
    if _p not in sys.path:
        sys.path.append(_p)

from contextlib import ExitStack

import numpy as np

import concourse.bacc as bacc
import concourse.mybir as mybir
import concourse.tile as tile
from concourse.bass import ts
from concourse.bass_utils import run_bass_kernel_spmd

B, T, C = 2, 2048, 1024
HQ, HKV, HD = 16, 2, 64
F32 = mybir.dt.float32
F32R = mybir.dt.float32r
AF = mybir.ActivationFunctionType
NCC = C // 128  # 8 chunks of the contraction dim
NEG = -1.0e30
SCALE = 1.0 / 64.0  # the reference's double 1/sqrt(64) scaling





def _emit(nc, tc, ctx, d):
    sing = ctx.enter_context(tc.tile_pool(name="sing", bufs=1))

    xT_sb = sing.tile([128, NCC, T], F32R)
    wq_sb = sing.tile([128, NCC, 256], F32R)
    wkv_sb = sing.tile([128, NCC, 128], F32R)
    wo_sb = sing.tile([128, 2, C], F32R)
    bq_sb = sing.tile([128, 2], F32)
    bkv_sb = sing.tile([128, 1], F32)
    cos_sb = sing.tile([128, T], F32)
    sin_sb = sing.tile([128, T], F32)
    r2t_sb = sing.tile([128, 128], F32R)
    id_sb = sing.tile([128, 128], F32R)
    dm_sb = sing.tile([128, 128], F32)
    bsel_sb = sing.tile([1, 2, 128], F32R)
    qT_sb = sing.tile([128, 2, T], F32R)   # pair j: head 2j at parts 0:64, 2j+1 at 64:128
    kvT_sb = sing.tile([128, T], F32R)     # v at parts 0:64, k (pre-rope) at 64:128
    kz0_sb = sing.tile([128, T], F32R)     # roped k at 0:64, zeros at 64:128
    kz1_sb = sing.tile([128, T], F32R)     # zeros at 0:64, roped k at 64:128
    vA_sb = sing.tile([128, 16, 128], F32R)  # v[k-chunk, :64] + ones col + zero pad
    yT_sb = sing.tile([128, 2, T], F32R)   # normalized attention out, pair layout

    # input DMAs
    for cc in range(NCC):
        nc.sync.dma_start(
            out=xT_sb[:, cc, :],
            in_=d["xT"].ap().rearrange("(cc p) t -> p cc t", p=128)[:, cc, :],
        )
    nc.sync.dma_start(out=wq_sb[:], in_=d["wq"].ap().rearrange("(cc p) m -> p cc m", p=128))
    nc.sync.dma_start(out=wkv_sb[:], in_=d["wkv"].ap().rearrange("(cc p) m -> p cc m", p=128))
    nc.sync.dma_start(out=wo_sb[:], in_=d["wo"].ap().rearrange("(j p) c -> p j c", p=128))
    nc.sync.dma_start(out=bq_sb[:], in_=d["bq"].ap())
    nc.sync.dma_start(out=bkv_sb[:], in_=d["bkv"].ap())
    nc.sync.dma_start(out=cos_sb[:], in_=d["cos2"].ap())
    nc.sync.dma_start(out=sin_sb[:], in_=d["sin2"].ap())
    nc.sync.dma_start(out=r2t_sb[:], in_=d["r2t"].ap())
    nc.sync.dma_start(out=id_sb[:], in_=d["ident"].ap())
    nc.sync.dma_start(out=dm_sb[:], in_=d["dmask"].ap())
    nc.sync.dma_start(out=bsel_sb[:], in_=d["bsel"].ap())

    # ---- phase 1: projections, RoPE, v transpose ----
    with tc.tile_pool(name="pp1", bufs=2, space="PSUM") as pp1, \
         tc.tile_pool(name="tmp1", bufs=2) as tmp1:
        # kv projection -> kvT_sb (v | k), with bias
        for ch in range(4):
            ps = pp1.tile([128, 512], F32, tag="proj")
            for cc in range(NCC):
                nc.tensor.matmul(
                    ps[:], wkv_sb[:, cc, :], xT_sb[:, cc, ts(ch, 512)],
                    start=(cc == 0), stop=(cc == NCC - 1),
                )
            nc.scalar.activation(
                out=kvT_sb[:, ts(ch, 512)], in_=ps[:],
                func=AF.Identity, bias=bkv_sb[:, 0:1], scale=1.0,
            )
        # v -> [Tk, 64] layout with ones column (for the denominator)
        for c16 in range(16):
            pv = pp1.tile([128, 64], F32R, tag="vt")
            nc.tensor.transpose(pv[:], kvT_sb[0:64, ts(c16, 128)], id_sb[0:64, 0:64])
            nc.vector.tensor_copy(vA_sb[:, c16, 0:64], pv[:])
        nc.vector.memset(vA_sb[:, :, 64:65].bitcast(F32), 1.0)
        nc.vector.memset(vA_sb[:, :, 65:128].bitcast(F32), 0.0)
        # RoPE on k (lives at partitions 64:128); fp32r matmuls must write
        # PSUM at base partition 0, so the rotation lands at 0:64 and the
        # roped k is assembled at 0:64 then duplicated up to 64:128
        for ch in range(4):
            pr = pp1.tile([128, 512], F32, tag="rot")
            nc.tensor.matmul(
                pr[0:64, :], r2t_sb[64:128, 64:128],
                kvT_sb[64:128, ts(ch, 512)], start=True, stop=True,
            )
            t1 = tmp1.tile([128, 512], F32, tag="t1")
            t2 = tmp1.tile([128, 512], F32, tag="t2")
            nc.vector.tensor_mul(t1[0:64, :], kvT_sb[64:128, ts(ch, 512)], cos_sb[64:128, ts(ch, 512)])
            nc.vector.tensor_mul(t2[0:64, :], pr[0:64, :], sin_sb[0:64, ts(ch, 512)])
            nc.vector.tensor_add(kz0_sb[0:64, ts(ch, 512)], t1[0:64, :], t2[0:64, :])
        nc.vector.memset(kz0_sb[64:128, :].bitcast(F32), 0.0)
        nc.vector.memset(kz1_sb[0:64, :].bitcast(F32), 0.0)
        nc.sync.dma_start(out=kz1_sb[64:128, :], in_=kz0_sb[0:64, :])
        # q projection + bias + RoPE (in pair layout)
        for j in range(2):
            for ch in range(4):
                ps = pp1.tile([128, 512], F32, tag="proj")
                for cc in range(NCC):
                    nc.tensor.matmul(
                        ps[:], wq_sb[:, cc, ts(j, 128)], xT_sb[:, cc, ts(ch, 512)],
                        start=(cc == 0), stop=(cc == NCC - 1),
                    )
                nc.scalar.activation(
                    out=qT_sb[:, j, ts(ch, 512)], in_=ps[:],
                    func=AF.Identity, bias=bq_sb[:, j:j + 1], scale=1.0,
                )
                pr = pp1.tile([128, 512], F32, tag="rot")
                nc.tensor.matmul(
                    pr[:], r2t_sb[:], qT_sb[:, j, ts(ch, 512)],
                    start=True, stop=True,
                )
                t1 = tmp1.tile([128, 512], F32, tag="t1")
                t2 = tmp1.tile([128, 512], F32, tag="t2")
                nc.vector.tensor_mul(t1[:], qT_sb[:, j, ts(ch, 512)], cos_sb[:, ts(ch, 512)])
                nc.vector.tensor_mul(t2[:], pr[:], sin_sb[:, ts(ch, 512)])
                nc.vector.tensor_add(qT_sb[:, j, ts(ch, 512)], t1[:], t2[:])

    # ---- phase 2: attention per head ----
    with tc.tile_pool(name="pps", bufs=4, space="PSUM") as pps, \
         tc.tile_pool(name="ppy", bufs=2, space="PSUM") as ppy, \
         tc.tile_pool(name="ppb", bufs=1, space="PSUM") as ppb, \
         tc.tile_pool(name="expp", bufs=3) as expp, \
         tc.tile_pool(name="nrm", bufs=2) as nrm:
        for qb in range(4):
            for h in range(4):
                j, b0 = h // 2, (h % 2) * 64
                py = ppy.tile([128, 512], F32, tag="y")
                kz = kz0_sb if h % 2 == 0 else kz1_sb
                for kb in range(4 * qb):
                    s_ = pps.tile([128, 512], F32, tag="s")
                    nc.tensor.matmul(
                        s_[:], kz[:, ts(kb, 128)],
                        qT_sb[:, j, ts(qb, 512)], start=True, stop=True,
                    )
                    e_ = expp.tile([128, 512], F32R, tag="e")
                    nc.scalar.activation(out=e_[:], in_=s_[:], func=AF.Exp, scale=SCALE)
                    nc.tensor.matmul(
                        py[:], vA_sb[:, kb, :], e_[:],
                        start=(kb == 0), stop=False,
                    )
                # diagonal band: k-chunk 4qb+r covers q in [kb*128, (qb+1)*512);
                # only its first 128 columns straddle the diagonal and get masked
                for r in range(4):
                    kb = 4 * qb + r
                    width = 512 - 128 * r
                    qoff = kb * 128
                    sd = pps.tile([128, 512], F32, tag="s")
                    nc.tensor.matmul(
                        sd[:, 0:width], kz[:, ts(kb, 128)],
                        qT_sb[:, j, qoff:qoff + width], start=True, stop=True,
                    )
                    nc.vector.tensor_add(sd[:, 0:128], sd[:, 0:128], dm_sb[:])
                    ed = expp.tile([128, 512], F32R, tag="e")
                    nc.scalar.activation(out=ed[:, 0:width], in_=sd[:, 0:width], func=AF.Exp, scale=SCALE)
                    # start/stop are bank-granular: start only on the very first
                    # matmul into py, stop only on the last
                    nc.tensor.matmul(
                        py[:, 128 * r:512], vA_sb[:, kb, :], ed[:, 0:width],
                        start=(qb == 0 and r == 0), stop=(r == 3),
                    )
                # normalize: reciprocal done in partition-major [128, 4] layout
                # (4 elems/lane instead of 512), broadcast back via a K=1 matmul
                dn = nrm.tile([1, 512], F32, tag="dn")
                nc.vector.tensor_copy(dn[0:1, :], py[64:65, :])
                nc.vector.tensor_copy(yT_sb[b0:b0 + 64, j, ts(qb, 512)], py[0:64, :])
                dnp = nrm.tile([128, 4], F32, tag="dnp")
                nc.sync.dma_start(out=dnp[:], in_=dn[0:1, :])
                rdp = nrm.tile([128, 4], F32, tag="rdp")
                nc.vector.reciprocal(rdp[:], dnp[:])
                rdr = nrm.tile([1, 512], F32R, tag="rdr")
                nc.sync.dma_start(out=rdr[0:1, :], in_=rdp[:].bitcast(F32R))
                pb = ppb.tile([128, 512], F32, tag="b")
                nc.tensor.matmul(
                    pb[:], bsel_sb[0:1, h % 2, :], rdr[0:1, :],
                    start=True, stop=True,
                )
                nc.vector.tensor_mul(
                    yT_sb[b0:b0 + 64, j, ts(qb, 512)],
                    yT_sb[b0:b0 + 64, j, ts(qb, 512)], pb[b0:b0 + 64, :],
                )

    # ---- phase 3: out projection ----
    with tc.tile_pool(name="ppo", bufs=4, space="PSUM") as ppo, \
         tc.tile_pool(name="ost", bufs=3) as ost:
        for tq in range(16):
            for cf in range(2):
                po = ppo.tile([128, 512], F32, tag="o")
                for j in range(2):
                    nc.tensor.matmul(
                        po[:], yT_sb[:, j, ts(tq, 128)], wo_sb[:, j, ts(cf, 512)],
                        start=(j == 0), stop=(j == 1),
                    )
                ob = ost.tile([128, 512], F32, tag="ob")
                nc.vector.tensor_copy(ob[:], po[:])
                nc.sync.dma_start(out=d["out"].ap()[ts(tq, 128), ts(cf, 512)], in_=ob[:])


def build_program():
    nc = bacc.Bacc("TRN2", target_bir_lowering=False, debug=False, num_devices=8)
    d = {}
    MM_IN = {"xT", "wq", "wkv", "wo", "r2t", "ident", "bsel"}
    for name, shape in [
        ("xT", [C, T]), ("wq", [C, 256]), ("wkv", [C, 128]),
        ("bq", [128, 2]), ("bkv", [128, 1]), ("wo", [256, C]),
        ("cos2", [128, T]), ("sin2", [128, T]), ("r2t", [128, 128]),
        ("ident", [128, 128]), ("dmask", [128, 128]), ("bsel", [1, 2, 128]),
    ]:
        dt = F32R if name in MM_IN else F32
        d[name] = nc.dram_tensor(name, shape, dt, kind="ExternalInput")
    d["out"] = nc.dram_tensor("out", [T, C], F32, kind="ExternalOutput")
    with tile.TileContext(nc) as tc, ExitStack() as ctx:
        _emit(nc, tc, ctx, d)
    nc.compile()
    return nc


def host_prep(inputs):
    """Slice/transpose the full inputs into the 8 per-core input maps."""
    f = lambda a: np.ascontiguousarray(np.asarray(a, dtype=np.float32))
    x, rc = f(inputs["x"]), f(inputs["rope_cache"])
    Wq, bq = f(inputs["Wq"]), f(inputs["bq"])
    Wk, bk = f(inputs["Wk"]), f(inputs["bk"])
    Wv, bv = f(inputs["Wv"]), f(inputs["bv"])
    Wo = f(inputs["Wo"])

    cos2 = np.tile(np.repeat(rc[:, 1::2].T, 2, axis=0), (2, 1))  # [128, T]
    sin2 = np.tile(np.repeat(rc[:, 0::2].T, 2, axis=0), (2, 1))
    R2 = np.zeros((128, 128), np.float32)
    for i in range(64):
        R2[2 * i, 2 * i + 1] = -1.0
        R2[2 * i + 1, 2 * i] = 1.0
    r2t = np.ascontiguousarray(R2.T)
    ident = np.eye(128, dtype=np.float32)
    kk, qq = np.arange(128)[:, None], np.arange(128)[None, :]
    dmask = np.where(kk <= qq, 0.0, NEG).astype(np.float32)
    bsel = np.zeros((1, 2, 128), np.float32)
    bsel[0, 0, 0:64] = 1.0
    bsel[0, 1, 64:128] = 1.0

    in_maps = []
    for core in range(8):
        b, g = core // 4, core % 4
        kv = g // 2
        in_maps.append({
            "xT": np.ascontiguousarray(x[b].T),
            "wq": np.ascontiguousarray(Wq[256 * g:256 * (g + 1), :].T),
            "wkv": np.ascontiguousarray(np.concatenate(
                [Wv[64 * kv:64 * (kv + 1)].T, Wk[64 * kv:64 * (kv + 1)].T], axis=1)),
            "bq": np.ascontiguousarray(bq[256 * g:256 * (g + 1)].reshape(2, 128).T),
            "bkv": np.concatenate(
                [bv[64 * kv:64 * (kv + 1)], bk[64 * kv:64 * (kv + 1)]]).reshape(128, 1),
            "wo": np.ascontiguousarray(Wo[:, 256 * g:256 * (g + 1)].T),
            "cos2": cos2, "sin2": sin2, "r2t": r2t,
            "ident": ident, "dmask": dmask, "bsel": bsel,
        })
    return in_maps


_PROGRAM = None


def _get_program():
    global _PROGRAM
    if _PROGRAM is None:
        _PROGRAM = build_program()
    return _PROGRAM


def _gather(results, bo):
    full = np.empty((B, T, C), np.float32)
    for b in range(B):
        acc = results[4 * b]["out"].astype(np.float32).copy()
        for g in range(1, 4):
            acc += results[4 * b + g]["out"]
        full[b] = acc + bo
    return full


def kernel(**inputs):
    nc = _get_program()
    in_maps = host_prep(inputs)
    res = run_bass_kernel_spmd(nc, in_maps, list(range(8)))
    return _gather(res.results, np.asarray(inputs["bo"], np.float32))


def kernel_traced(**inputs):
    """Like kernel() but with NTFF tracing; returns (output, BassKernelResults)."""
    nc = _get_program()
    in_maps = host_prep(inputs)
    res = run_bass_kernel_spmd(nc, in_maps, list(range(8)), trace=True)
    return _gather(res.results, np.asarray(inputs["bo"], np.float32)), res


# revision 14
# speedup vs baseline: 1.6255x; 1.0385x over previous
"""Causal self-attention (GQA + RoPE) on 8 Trainium2 NeuronCores.

Sharding: core c = (b, g) with b = c // 4 (batch), g = c % 4 (group of 4
consecutive Q heads; KV head g // 2). Each core computes the attention
output for its 4 heads and a partial out-projection through the matching
256-column slice of Wo. Host sums the 4 partials per batch and adds bo.

Per-core kernel (all activations kept in transposed [feature, T] layout):
  - q/k/v projections as fp32r matmuls contracting C on partitions
  - RoPE as x*cos + (R@x)*sin where R is a pair-rotation matrix (PE matmul)
  - scores computed pre-transposed sT[k, q] so softmax exp (ScalarE) lands
    directly in the layout the AV matmul needs; no PE transposes of exp
  - softmax denominator via a ones-column appended to V (M=65 AV matmul)
  - causal structure handled block-wise: full 128x512 blocks below the
    diagonal, masked 128x128 blocks on the diagonal
  - normalization (1/den) applied while evacuating the AV PSUM
"""

import sys

for _p in ("/opt/trn_rl_repo", "/opt/pypackages"):
    if _p not in sys.path:
        sys.path.append(_p)

from contextlib import ExitStack

import numpy as np

import concourse.bacc as bacc
import concourse.mybir as mybir
import concourse.tile as tile
from concourse.bass import ts
from concourse.bass_utils import run_bass_kernel_spmd

B, T, C = 2, 2048, 1024
HQ, HKV, HD = 16, 2, 64
F32 = mybir.dt.float32
F32R = mybir.dt.float32r
AF = mybir.ActivationFunctionType
NCC = C // 128  # 8 chunks of the contraction dim
NEG = -1.0e30
SCALE = 1.0 / 64.0  # the reference's double 1/sqrt(64) scaling





def _emit(nc, tc, ctx, d):
    sing = ctx.enter_context(tc.tile_pool(name="sing", bufs=1))

    xT_sb = sing.tile([128, NCC, T], F32R)
    wq_sb = sing.tile([128, NCC, 256], F32R)
    wkv_sb = sing.tile([128, NCC, 128], F32R)
    wo_sb = sing.tile([128, 2, C], F32R)
    bq_sb = sing.tile([128, 2], F32)
    bkv_sb = sing.tile([128, 1], F32)
    cos_sb = sing.tile([128, T], F32)
    sin_sb = sing.tile([128, T], F32)
    r2t_sb = sing.tile([128, 128], F32R)
    id_sb = sing.tile([128, 128], F32R)
    dm_sb = sing.tile([128, 128], F32)
    bsel_sb = sing.tile([1, 2, 128], F32R)
    qT_sb = sing.tile([128, 2, T], F32R)   # pair j: head 2j at parts 0:64, 2j+1 at 64:128
    kvT_sb = sing.tile([128, T], F32R)     # v at parts 0:64, k (pre-rope) at 64:128
    kz0_sb = sing.tile([128, T], F32R)     # roped k at 0:64, zeros at 64:128
    kz1_sb = sing.tile([128, T], F32R)     # zeros at 0:64, roped k at 64:128
    vA_sb = sing.tile([128, 16, 128], F32R)  # v[k-chunk, :64] + ones col + zero pad
    yT_sb = sing.tile([128, 2, T], F32R)   # normalized attention out, pair layout

    # input DMAs: small weights/tables first so the first projection
    # matmuls start as early as possible; x streams in behind them;
    # wo (needed only by the out-projection) goes last
    nc.sync.dma_start(out=wkv_sb[:], in_=d["wkv"].ap().rearrange("(cc p) m -> p cc m", p=128))
    nc.sync.dma_start(out=bkv_sb[:], in_=d["bkv"].ap())
    nc.sync.dma_start(out=wq_sb[:], in_=d["wq"].ap().rearrange("(cc p) m -> p cc m", p=128))
    nc.sync.dma_start(out=bq_sb[:], in_=d["bq"].ap())
    nc.sync.dma_start(out=id_sb[:], in_=d["ident"].ap())
    nc.sync.dma_start(out=cos_sb[:], in_=d["cos2"].ap())
    nc.sync.dma_start(out=sin_sb[:], in_=d["sin2"].ap())
    nc.sync.dma_start(out=r2t_sb[:], in_=d["r2t"].ap())
    nc.sync.dma_start(out=dm_sb[:], in_=d["dmask"].ap())
    nc.sync.dma_start(out=bsel_sb[:], in_=d["bsel"].ap())
    for cc in range(NCC):
        nc.sync.dma_start(
            out=xT_sb[:, cc, :],
            in_=d["xT"].ap().rearrange("(cc p) t -> p cc t", p=128)[:, cc, :],
        )
    nc.sync.dma_start(out=wo_sb[:], in_=d["wo"].ap().rearrange("(j p) c -> p j c", p=128))

    # ---- phase 1: projections, RoPE, v transpose ----
    with tc.tile_pool(name="pp1", bufs=2, space="PSUM") as pp1, \
         tc.tile_pool(name="tmp1", bufs=2) as tmp1:
        # kv projection -> kvT_sb (v | k), with bias
        for ch in range(4):
            ps = pp1.tile([128, 512], F32, tag="proj")
            for cc in range(NCC):
                nc.tensor.matmul(
                    ps[:], wkv_sb[:, cc, :], xT_sb[:, cc, ts(ch, 512)],
                    start=(cc == 0), stop=(cc == NCC - 1),
                )
            nc.scalar.activation(
                out=kvT_sb[:, ts(ch, 512)], in_=ps[:],
                func=AF.Identity, bias=bkv_sb[:, 0:1], scale=1.0,
            )
        # v -> [Tk, 64] layout with ones column (for the denominator)
        for c16 in range(16):
            pv = pp1.tile([128, 64], F32R, tag="vt")
            nc.tensor.transpose(pv[:], kvT_sb[0:64, ts(c16, 128)], id_sb[0:64, 0:64])
            nc.vector.tensor_copy(vA_sb[:, c16, 0:64], pv[:])
        nc.vector.memset(vA_sb[:, :, 64:65].bitcast(F32), 1.0)
        nc.vector.memset(vA_sb[:, :, 65:128].bitcast(F32), 0.0)
        # RoPE on k (lives at partitions 64:128); fp32r matmuls must write
        # PSUM at base partition 0, so the rotation lands at 0:64 and the
        # roped k is assembled at 0:64 then duplicated up to 64:128
        for ch in range(4):
            pr = pp1.tile([128, 512], F32, tag="rot")
            nc.tensor.matmul(
                pr[0:64, :], r2t_sb[64:128, 64:128],
                kvT_sb[64:128, ts(ch, 512)], start=True, stop=True,
            )
            t1 = tmp1.tile([128, 512], F32, tag="t1")
            t2 = tmp1.tile([128, 512], F32, tag="t2")
            nc.vector.tensor_mul(t1[0:64, :], kvT_sb[64:128, ts(ch, 512)], cos_sb[64:128, ts(ch, 512)])
            nc.vector.tensor_mul(t2[0:64, :], pr[0:64, :], sin_sb[0:64, ts(ch, 512)])
            nc.vector.tensor_add(kz0_sb[0:64, ts(ch, 512)], t1[0:64, :], t2[0:64, :])
        nc.vector.memset(kz0_sb[64:128, :].bitcast(F32), 0.0)
        nc.vector.memset(kz1_sb[0:64, :].bitcast(F32), 0.0)
        nc.sync.dma_start(out=kz1_sb[64:128, :], in_=kz0_sb[0:64, :])
        # q projection + bias + RoPE (in pair layout)
        for j in range(2):
            for ch in range(4):
                ps = pp1.tile([128, 512], F32, tag="proj")
                for cc in range(NCC):
                    nc.tensor.matmul(
                        ps[:], wq_sb[:, cc, ts(j, 128)], xT_sb[:, cc, ts(ch, 512)],
                        start=(cc == 0), stop=(cc == NCC - 1),
                    )
                nc.scalar.activation(
                    out=qT_sb[:, j, ts(ch, 512)], in_=ps[:],
                    func=AF.Identity, bias=bq_sb[:, j:j + 1], scale=1.0,
                )
                pr = pp1.tile([128, 512], F32, tag="rot")
                nc.tensor.matmul(
                    pr[:], r2t_sb[:], qT_sb[:, j, ts(ch, 512)],
                    start=True, stop=True,
                )
                t1 = tmp1.tile([128, 512], F32, tag="t1")
                t2 = tmp1.tile([128, 512], F32, tag="t2")
                nc.vector.tensor_mul(t1[:], qT_sb[:, j, ts(ch, 512)], cos_sb[:, ts(ch, 512)])
                nc.vector.tensor_mul(t2[:], pr[:], sin_sb[:, ts(ch, 512)])
                nc.vector.tensor_add(qT_sb[:, j, ts(ch, 512)], t1[:], t2[:])

    # ---- phase 2: attention per head ----
    with tc.tile_pool(name="pps", bufs=3, space="PSUM") as pps, \
         tc.tile_pool(name="ppy", bufs=2, space="PSUM") as ppy, \
         tc.tile_pool(name="ppb", bufs=1, space="PSUM") as ppb, \
         tc.tile_pool(name="ppo", bufs=2, space="PSUM") as ppo, \
         tc.tile_pool(name="expp", bufs=3) as expp, \
         tc.tile_pool(name="ost", bufs=3) as ost, \
         tc.tile_pool(name="nrm", bufs=2) as nrm:
        for qb in range(4):
            for h in range(4):
                j, b0 = h // 2, (h % 2) * 64
                py = ppy.tile([128, 512], F32, tag="y")
                kz = kz0_sb if h % 2 == 0 else kz1_sb
                for kb in range(4 * qb):
                    s_ = pps.tile([128, 512], F32, tag="s")
                    nc.tensor.matmul(
                        s_[:], kz[:, ts(kb, 128)],
                        qT_sb[:, j, ts(qb, 512)], start=True, stop=True,
                    )
                    e_ = expp.tile([128, 512], F32R, tag="e")
                    nc.scalar.activation(out=e_[:], in_=s_[:], func=AF.Exp, scale=SCALE)
                    nc.tensor.matmul(
                        py[:], vA_sb[:, kb, :], e_[:],
                        start=(kb == 0), stop=False,
                    )
                # diagonal band: k-chunk 4qb+r covers q in [kb*128, (qb+1)*512);
                # only its first 128 columns straddle the diagonal and get masked
                for r in range(4):
                    kb = 4 * qb + r
                    width = 512 - 128 * r
                    qoff = kb * 128
                    sd = pps.tile([128, 512], F32, tag="s")
                    nc.tensor.matmul(
                        sd[:, 0:width], kz[:, ts(kb, 128)],
                        qT_sb[:, j, qoff:qoff + width], start=True, stop=True,
                    )
                    nc.vector.tensor_add(sd[:, 0:128], sd[:, 0:128], dm_sb[:])
                    ed = expp.tile([128, 512], F32R, tag="e")
                    nc.scalar.activation(out=ed[:, 0:width], in_=sd[:, 0:width], func=AF.Exp, scale=SCALE)
                    # start/stop are bank-granular: start only on the very first
                    # matmul into py, stop only on the last
                    nc.tensor.matmul(
                        py[:, 128 * r:512], vA_sb[:, kb, :], ed[:, 0:width],
                        start=(qb == 0 and r == 0), stop=(r == 3),
                    )
                # normalize: reciprocal done in partition-major [128, 4] layout
                # (4 elems/lane instead of 512), broadcast back via a K=1 matmul
                dn = nrm.tile([1, 512], F32, tag="dn")
                nc.vector.tensor_copy(dn[0:1, :], py[64:65, :])
                nc.vector.tensor_copy(yT_sb[b0:b0 + 64, j, ts(qb, 512)], py[0:64, :])
                dnp = nrm.tile([128, 4], F32, tag="dnp")
                nc.sync.dma_start(out=dnp[:], in_=dn[0:1, :])
                rdp = nrm.tile([128, 4], F32, tag="rdp")
                nc.vector.reciprocal(rdp[:], dnp[:])
                rdr = nrm.tile([1, 512], F32R, tag="rdr")
                nc.sync.dma_start(out=rdr[0:1, :], in_=rdp[:].bitcast(F32R))
                pb = ppb.tile([128, 512], F32, tag="b")
                nc.tensor.matmul(
                    pb[:], bsel_sb[0:1, h % 2, :], rdr[0:1, :],
                    start=True, stop=True,
                )
                nc.vector.tensor_mul(
                    yT_sb[b0:b0 + 64, j, ts(qb, 512)],
                    yT_sb[b0:b0 + 64, j, ts(qb, 512)], pb[b0:b0 + 64, :],
                )
            # out projection for this q-block (all 4 heads now normalized)
            for tq in range(4 * qb, 4 * qb + 4):
                for cf in range(2):
                    po = ppo.tile([128, 512], F32, tag="o")
                    for j in range(2):
                        nc.tensor.matmul(
                            po[:], yT_sb[:, j, ts(tq, 128)], wo_sb[:, j, ts(cf, 512)],
                            start=(j == 0), stop=(j == 1),
                        )
                    ob = ost.tile([128, 512], F32, tag="ob")
                    nc.vector.tensor_copy(ob[:], po[:])
                    nc.sync.dma_start(out=d["out"].ap()[ts(tq, 128), ts(cf, 512)], in_=ob[:])


def build_program():
    nc = bacc.Bacc("TRN2", target_bir_lowering=False, debug=False, num_devices=8)
    d = {}
    MM_IN = {"xT", "wq", "wkv", "wo", "r2t", "ident", "bsel"}
    for name, shape in [
        ("xT", [C, T]), ("wq", [C, 256]), ("wkv", [C, 128]),
        ("bq", [128, 2]), ("bkv", [128, 1]), ("wo", [256, C]),
        ("cos2", [128, T]), ("sin2", [128, T]), ("r2t", [128, 128]),
        ("ident", [128, 128]), ("dmask", [128, 128]), ("bsel", [1, 2, 128]),
    ]:
        dt = F32R if name in MM_IN else F32
        d[name] = nc.dram_tensor(name, shape, dt, kind="ExternalInput")
    d["out"] = nc.dram_tensor("out", [T, C], F32, kind="ExternalOutput")
    with tile.TileContext(nc) as tc, ExitStack() as ctx:
        _emit(nc, tc, ctx, d)
    nc.compile()
    return nc


def host_prep(inputs):
    """Slice/transpose the full inputs into the 8 per-core input maps."""
    f = lambda a: np.ascontiguousarray(np.asarray(a, dtype=np.float32))
    x, rc = f(inputs["x"]), f(inputs["rope_cache"])
    Wq, bq = f(inputs["Wq"]), f(inputs["bq"])
    Wk, bk = f(inputs["Wk"]), f(inputs["bk"])
    Wv, bv = f(inputs["Wv"]), f(inputs["bv"])
    Wo = f(inputs["Wo"])

    cos2 = np.tile(np.repeat(rc[:, 1::2].T, 2, axis=0), (2, 1))  # [128, T]
    sin2 = np.tile(np.repeat(rc[:, 0::2].T, 2, axis=0), (2, 1))
    R2 = np.zeros((128, 128), np.float32)
    for i in range(64):
        R2[2 * i, 2 * i + 1] = -1.0
        R2[2 * i + 1, 2 * i] = 1.0
    r2t = np.ascontiguousarray(R2.T)
    ident = np.eye(128, dtype=np.float32)
    kk, qq = np.arange(128)[:, None], np.arange(128)[None, :]
    dmask = np.where(kk <= qq, 0.0, NEG).astype(np.float32)
    bsel = np.zeros((1, 2, 128), np.float32)
    bsel[0, 0, 0:64] = 1.0
    bsel[0, 1, 64:128] = 1.0

    in_maps = []
    for core in range(8):
        b, g = core // 4, core % 4
        kv = g // 2
        in_maps.append({
            "xT": np.ascontiguousarray(x[b].T),
            "wq": np.ascontiguousarray(Wq[256 * g:256 * (g + 1), :].T),
            "wkv": np.ascontiguousarray(np.concatenate(
                [Wv[64 * kv:64 * (kv + 1)].T, Wk[64 * kv:64 * (kv + 1)].T], axis=1)),
            "bq": np.ascontiguousarray(bq[256 * g:256 * (g + 1)].reshape(2, 128).T),
            "bkv": np.concatenate(
                [bv[64 * kv:64 * (kv + 1)], bk[64 * kv:64 * (kv + 1)]]).reshape(128, 1),
            "wo": np.ascontiguousarray(Wo[:, 256 * g:256 * (g + 1)].T),
            "cos2": cos2, "sin2": sin2, "r2t": r2t,
            "ident": ident, "dmask": dmask, "bsel": bsel,
        })
    return in_maps


_PROGRAM = None


def _get_program():
    global _PROGRAM
    if _PROGRAM is None:
        _PROGRAM = build_program()
    return _PROGRAM


def _gather(results, bo):
    full = np.empty((B, T, C), np.float32)
    for b in range(B):
        acc = results[4 * b]["out"].astype(np.float32).copy()
        for g in range(1, 4):
            acc += results[4 * b + g]["out"]
        full[b] = acc + bo
    return full


def kernel(**inputs):
    nc = _get_program()
    in_maps = host_prep(inputs)
    res = run_bass_kernel_spmd(nc, in_maps, list(range(8)))
    return _gather(res.results, np.asarray(inputs["bo"], np.float32))


def kernel_traced(**inputs):
    """Like kernel() but with NTFF tracing; returns (output, BassKernelResults)."""
    nc = _get_program()
    in_maps = host_prep(inputs)
    res = run_bass_kernel_spmd(nc, in_maps, list(range(8)), trace=True)
    return _gather(res.results, np.asarray(inputs["bo"], np.float32)), res


# revision 15
# speedup vs baseline: 1.6652x; 1.0244x over previous
"""Causal self-attention (GQA + RoPE) on 8 Trainium2 NeuronCores.

Sharding: core c = (b, g) with b = c // 4 (batch), g = c % 4 (group of 4
consecutive Q heads; KV head g // 2). Each core computes the attention
output for its 4 heads and a partial out-projection through the matching
256-column slice of Wo. Host sums the 4 partials per batch and adds bo.

Per-core kernel (all activations kept in transposed [feature, T] layout):
  - q/k/v projections as fp32r matmuls contracting C on partitions
  - RoPE as x*cos + (R@x)*sin where R is a pair-rotation matrix (PE matmul)
  - scores computed pre-transposed sT[k, q] so softmax exp (ScalarE) lands
    directly in the layout the AV matmul needs; no PE transposes of exp
  - softmax denominator via a ones-column appended to V (M=65 AV matmul)
  - causal structure handled block-wise: full 128x512 blocks below the
    diagonal, masked 128x128 blocks on the diagonal
  - normalization (1/den) applied while evacuating the AV PSUM
"""

import sys

for _p in ("/opt/trn_rl_repo", "/opt/pypackages"):
    if _p not in sys.path:
        sys.path.append(_p)

from contextlib import ExitStack

import numpy as np

import concourse.bacc as bacc
import concourse.mybir as mybir
import concourse.tile as tile
from concourse.bass import ts
from concourse.bass_utils import run_bass_kernel_spmd

B, T, C = 2, 2048, 1024
HQ, HKV, HD = 16, 2, 64
F32 = mybir.dt.float32
F32R = mybir.dt.float32r
BF16 = mybir.dt.bfloat16
ATT_BF16 = True  # attention matmuls (QK/AV) in bf16; projections stay fp32r
ADT = BF16 if ATT_BF16 else F32R
AF = mybir.ActivationFunctionType
NCC = C // 128  # 8 chunks of the contraction dim
NEG = -1.0e30
SCALE = 1.0 / 64.0  # the reference's double 1/sqrt(64) scaling





def _emit(nc, tc, ctx, d):
    sing = ctx.enter_context(tc.tile_pool(name="sing", bufs=1))

    xT_sb = sing.tile([128, NCC, T], F32R)
    wq_sb = sing.tile([128, NCC, 256], F32R)
    wkv_sb = sing.tile([128, NCC, 128], F32R)
    wo_sb = sing.tile([128, 2, C], F32R)
    bq_sb = sing.tile([128, 2], F32)
    bkv_sb = sing.tile([128, 1], F32)
    cos_sb = sing.tile([128, T], F32)
    sin_sb = sing.tile([128, T], F32)
    r2t_sb = sing.tile([128, 128], ADT)
    id_sb = sing.tile([128, 128], ADT)
    dm_sb = sing.tile([128, 128], F32)
    bsel_sb = sing.tile([1, 2, 128], F32R)
    qT_sb = sing.tile([128, 2, T], ADT)   # pair j: head 2j at parts 0:64, 2j+1 at 64:128
    kvT_sb = sing.tile([128, T], ADT)     # v at parts 0:64, k (pre-rope) at 64:128
    kz0_sb = sing.tile([128, T], ADT)     # roped k at 0:64, zeros at 64:128
    kz1_sb = sing.tile([128, T], ADT)     # zeros at 0:64, roped k at 64:128
    vA_sb = sing.tile([128, 16, 128], ADT)  # v[k-chunk, :64] + ones col + zero pad
    yT_sb = sing.tile([128, 2, T], F32R)   # normalized attention out, pair layout

    # input DMAs: small weights/tables first so the first projection
    # matmuls start as early as possible; x streams in behind them;
    # wo (needed only by the out-projection) goes last
    nc.sync.dma_start(out=wkv_sb[:], in_=d["wkv"].ap().rearrange("(cc p) m -> p cc m", p=128))
    nc.sync.dma_start(out=bkv_sb[:], in_=d["bkv"].ap())
    nc.sync.dma_start(out=wq_sb[:], in_=d["wq"].ap().rearrange("(cc p) m -> p cc m", p=128))
    nc.sync.dma_start(out=bq_sb[:], in_=d["bq"].ap())
    nc.sync.dma_start(out=id_sb[:], in_=d["ident"].ap())
    nc.sync.dma_start(out=cos_sb[:], in_=d["cos2"].ap())
    nc.sync.dma_start(out=sin_sb[:], in_=d["sin2"].ap())
    nc.sync.dma_start(out=r2t_sb[:], in_=d["r2t"].ap())
    nc.sync.dma_start(out=dm_sb[:], in_=d["dmask"].ap())
    nc.sync.dma_start(out=bsel_sb[:], in_=d["bsel"].ap())
    for cc in range(NCC):
        nc.sync.dma_start(
            out=xT_sb[:, cc, :],
            in_=d["xT"].ap().rearrange("(cc p) t -> p cc t", p=128)[:, cc, :],
        )
    nc.sync.dma_start(out=wo_sb[:], in_=d["wo"].ap().rearrange("(j p) c -> p j c", p=128))

    # ---- phase 1: projections, RoPE, v transpose ----
    with tc.tile_pool(name="pp1", bufs=2, space="PSUM") as pp1, \
         tc.tile_pool(name="tmp1", bufs=2) as tmp1:
        # kv projection -> kvT_sb (v | k), with bias
        for ch in range(4):
            ps = pp1.tile([128, 512], F32, tag="proj")
            for cc in range(NCC):
                nc.tensor.matmul(
                    ps[:], wkv_sb[:, cc, :], xT_sb[:, cc, ts(ch, 512)],
                    start=(cc == 0), stop=(cc == NCC - 1),
                )
            nc.scalar.activation(
                out=kvT_sb[:, ts(ch, 512)], in_=ps[:],
                func=AF.Identity, bias=bkv_sb[:, 0:1], scale=1.0,
            )
        # v -> [Tk, 64] layout with ones column (for the denominator)
        for c16 in range(16):
            pv = pp1.tile([128, 64], ADT, tag="vt")
            nc.tensor.transpose(pv[:], kvT_sb[0:64, ts(c16, 128)], id_sb[0:64, 0:64])
            nc.vector.tensor_copy(vA_sb[:, c16, 0:64], pv[:])
        nc.vector.memset(vA_sb[:, :, 64:65] if ATT_BF16 else vA_sb[:, :, 64:65].bitcast(F32), 1.0)
        nc.vector.memset(vA_sb[:, :, 65:128] if ATT_BF16 else vA_sb[:, :, 65:128].bitcast(F32), 0.0)
        # RoPE on k (lives at partitions 64:128); fp32r matmuls must write
        # PSUM at base partition 0, so the rotation lands at 0:64 and the
        # roped k is assembled at 0:64 then duplicated up to 64:128
        for ch in range(4):
            pr = pp1.tile([128, 512], F32, tag="rot")
            nc.tensor.matmul(
                pr[0:64, :], r2t_sb[64:128, 64:128],
                kvT_sb[64:128, ts(ch, 512)], start=True, stop=True,
            )
            t1 = tmp1.tile([128, 512], F32, tag="t1")
            t2 = tmp1.tile([128, 512], F32, tag="t2")
            nc.vector.tensor_mul(t1[0:64, :], kvT_sb[64:128, ts(ch, 512)], cos_sb[64:128, ts(ch, 512)])
            nc.vector.tensor_mul(t2[0:64, :], pr[0:64, :], sin_sb[0:64, ts(ch, 512)])
            nc.vector.tensor_add(kz0_sb[0:64, ts(ch, 512)], t1[0:64, :], t2[0:64, :])
        nc.vector.memset(kz0_sb[64:128, :] if ATT_BF16 else kz0_sb[64:128, :].bitcast(F32), 0.0)
        nc.vector.memset(kz1_sb[0:64, :] if ATT_BF16 else kz1_sb[0:64, :].bitcast(F32), 0.0)
        nc.sync.dma_start(out=kz1_sb[64:128, :], in_=kz0_sb[0:64, :])
        # q projection + bias + RoPE (in pair layout)
        for j in range(2):
            for ch in range(4):
                ps = pp1.tile([128, 512], F32, tag="proj")
                for cc in range(NCC):
                    nc.tensor.matmul(
                        ps[:], wq_sb[:, cc, ts(j, 128)], xT_sb[:, cc, ts(ch, 512)],
                        start=(cc == 0), stop=(cc == NCC - 1),
                    )
                nc.scalar.activation(
                    out=qT_sb[:, j, ts(ch, 512)], in_=ps[:],
                    func=AF.Identity, bias=bq_sb[:, j:j + 1], scale=1.0,
                )
                pr = pp1.tile([128, 512], F32, tag="rot")
                nc.tensor.matmul(
                    pr[:], r2t_sb[:], qT_sb[:, j, ts(ch, 512)],
                    start=True, stop=True,
                )
                t1 = tmp1.tile([128, 512], F32, tag="t1")
                t2 = tmp1.tile([128, 512], F32, tag="t2")
                nc.vector.tensor_mul(t1[:], qT_sb[:, j, ts(ch, 512)], cos_sb[:, ts(ch, 512)])
                nc.vector.tensor_mul(t2[:], pr[:], sin_sb[:, ts(ch, 512)])
                nc.vector.tensor_add(qT_sb[:, j, ts(ch, 512)], t1[:], t2[:])

    # ---- phase 2: attention per head ----
    with tc.tile_pool(name="pps", bufs=3, space="PSUM") as pps, \
         tc.tile_pool(name="ppy", bufs=2, space="PSUM") as ppy, \
         tc.tile_pool(name="ppb", bufs=1, space="PSUM") as ppb, \
         tc.tile_pool(name="ppo", bufs=2, space="PSUM") as ppo, \
         tc.tile_pool(name="expp", bufs=3) as expp, \
         tc.tile_pool(name="ost", bufs=3) as ost, \
         tc.tile_pool(name="nrm", bufs=2) as nrm:
        for qb in range(4):
            for h in range(4):
                j, b0 = h // 2, (h % 2) * 64
                py = ppy.tile([128, 512], F32, tag="y")
                kz = kz0_sb if h % 2 == 0 else kz1_sb
                for kb in range(4 * qb):
                    s_ = pps.tile([128, 512], F32, tag="s")
                    nc.tensor.matmul(
                        s_[:], kz[:, ts(kb, 128)],
                        qT_sb[:, j, ts(qb, 512)], start=True, stop=True,
                    )
                    e_ = expp.tile([128, 512], ADT, tag="e")
                    nc.scalar.activation(out=e_[:], in_=s_[:], func=AF.Exp, scale=SCALE)
                    nc.tensor.matmul(
                        py[:], vA_sb[:, kb, :], e_[:],
                        start=(kb == 0), stop=False,
                    )
                # diagonal band: k-chunk 4qb+r covers q in [kb*128, (qb+1)*512);
                # only its first 128 columns straddle the diagonal and get masked
                for r in range(4):
                    kb = 4 * qb + r
                    width = 512 - 128 * r
                    qoff = kb * 128
                    sd = pps.tile([128, 512], F32, tag="s")
                    nc.tensor.matmul(
                        sd[:, 0:width], kz[:, ts(kb, 128)],
                        qT_sb[:, j, qoff:qoff + width], start=True, stop=True,
                    )
                    nc.vector.tensor_add(sd[:, 0:128], sd[:, 0:128], dm_sb[:])
                    ed = expp.tile([128, 512], ADT, tag="e")
                    nc.scalar.activation(out=ed[:, 0:width], in_=sd[:, 0:width], func=AF.Exp, scale=SCALE)
                    # start/stop are bank-granular: start only on the very first
                    # matmul into py, stop only on the last
                    nc.tensor.matmul(
                        py[:, 128 * r:512], vA_sb[:, kb, :], ed[:, 0:width],
                        start=(qb == 0 and r == 0), stop=(r == 3),
                    )
                # normalize: reciprocal done in partition-major [128, 4] layout
                # (4 elems/lane instead of 512), broadcast back via a K=1 matmul
                dn = nrm.tile([1, 512], F32, tag="dn")
                nc.vector.tensor_copy(dn[0:1, :], py[64:65, :])
                nc.vector.tensor_copy(yT_sb[b0:b0 + 64, j, ts(qb, 512)], py[0:64, :])
                dnp = nrm.tile([128, 4], F32, tag="dnp")
                nc.sync.dma_start(out=dnp[:], in_=dn[0:1, :])
                rdp = nrm.tile([128, 4], F32, tag="rdp")
                nc.vector.reciprocal(rdp[:], dnp[:])
                rdr = nrm.tile([1, 512], F32R, tag="rdr")
                nc.sync.dma_start(out=rdr[0:1, :], in_=rdp[:].bitcast(F32R))
                pb = ppb.tile([128, 512], F32, tag="b")
                nc.tensor.matmul(
                    pb[:], bsel_sb[0:1, h % 2, :], rdr[0:1, :],
                    start=True, stop=True,
                )
                nc.vector.tensor_mul(
                    yT_sb[b0:b0 + 64, j, ts(qb, 512)],
                    yT_sb[b0:b0 + 64, j, ts(qb, 512)], pb[b0:b0 + 64, :],
                )
            # out projection for this q-block (all 4 heads now normalized)
            for tq in range(4 * qb, 4 * qb + 4):
                for cf in range(2):
                    po = ppo.tile([128, 512], F32, tag="o")
                    for j in range(2):
                        nc.tensor.matmul(
                            po[:], yT_sb[:, j, ts(tq, 128)], wo_sb[:, j, ts(cf, 512)],
                            start=(j == 0), stop=(j == 1),
                        )
                    ob = ost.tile([128, 512], F32, tag="ob")
                    nc.vector.tensor_copy(ob[:], po[:])
                    nc.sync.dma_start(out=d["out"].ap()[ts(tq, 128), ts(cf, 512)], in_=ob[:])


def build_program():
    nc = bacc.Bacc("TRN2", target_bir_lowering=False, debug=False, num_devices=8)
    d = {}
    MM_IN = {"xT", "wq", "wkv", "wo", "bsel"}
    ATT_IN = {"r2t", "ident"}
    for name, shape in [
        ("xT", [C, T]), ("wq", [C, 256]), ("wkv", [C, 128]),
        ("bq", [128, 2]), ("bkv", [128, 1]), ("wo", [256, C]),
        ("cos2", [128, T]), ("sin2", [128, T]), ("r2t", [128, 128]),
        ("ident", [128, 128]), ("dmask", [128, 128]), ("bsel", [1, 2, 128]),
    ]:
        dt = F32R if name in MM_IN else (ADT if name in ATT_IN else F32)
        d[name] = nc.dram_tensor(name, shape, dt, kind="ExternalInput")
    d["out"] = nc.dram_tensor("out", [T, C], F32, kind="ExternalOutput")
    with tile.TileContext(nc) as tc, ExitStack() as ctx:
        _emit(nc, tc, ctx, d)
    nc.compile()
    return nc


def host_prep(inputs):
    """Slice/transpose the full inputs into the 8 per-core input maps."""
    if ATT_BF16:
        import ml_dtypes
        _att = lambda a: a.astype(ml_dtypes.bfloat16)
    else:
        _att = lambda a: a
    f = lambda a: np.ascontiguousarray(np.asarray(a, dtype=np.float32))
    x, rc = f(inputs["x"]), f(inputs["rope_cache"])
    Wq, bq = f(inputs["Wq"]), f(inputs["bq"])
    Wk, bk = f(inputs["Wk"]), f(inputs["bk"])
    Wv, bv = f(inputs["Wv"]), f(inputs["bv"])
    Wo = f(inputs["Wo"])

    cos2 = np.tile(np.repeat(rc[:, 1::2].T, 2, axis=0), (2, 1))  # [128, T]
    sin2 = np.tile(np.repeat(rc[:, 0::2].T, 2, axis=0), (2, 1))
    R2 = np.zeros((128, 128), np.float32)
    for i in range(64):
        R2[2 * i, 2 * i + 1] = -1.0
        R2[2 * i + 1, 2 * i] = 1.0
    r2t = np.ascontiguousarray(R2.T)
    ident = np.eye(128, dtype=np.float32)
    kk, qq = np.arange(128)[:, None], np.arange(128)[None, :]
    dmask = np.where(kk <= qq, 0.0, NEG).astype(np.float32)
    bsel = np.zeros((1, 2, 128), np.float32)
    bsel[0, 0, 0:64] = 1.0
    bsel[0, 1, 64:128] = 1.0

    in_maps = []
    for core in range(8):
        b, g = core // 4, core % 4
        kv = g // 2
        in_maps.append({
            "xT": np.ascontiguousarray(x[b].T),
            "wq": np.ascontiguousarray(Wq[256 * g:256 * (g + 1), :].T),
            "wkv": np.ascontiguousarray(np.concatenate(
                [Wv[64 * kv:64 * (kv + 1)].T, Wk[64 * kv:64 * (kv + 1)].T], axis=1)),
            "bq": np.ascontiguousarray(bq[256 * g:256 * (g + 1)].reshape(2, 128).T),
            "bkv": np.concatenate(
                [bv[64 * kv:64 * (kv + 1)], bk[64 * kv:64 * (kv + 1)]]).reshape(128, 1),
            "wo": np.ascontiguousarray(Wo[:, 256 * g:256 * (g + 1)].T),
            "cos2": cos2, "sin2": sin2, "r2t": _att(r2t),
            "ident": _att(ident), "dmask": dmask, "bsel": bsel,
        })
    return in_maps


_PROGRAM = None


def _get_program():
    global _PROGRAM
    if _PROGRAM is None:
        _PROGRAM = build_program()
    return _PROGRAM


def _gather(results, bo):
    full = np.empty((B, T, C), np.float32)
    for b in range(B):
        acc = results[4 * b]["out"].astype(np.float32).copy()
        for g in range(1, 4):
            acc += results[4 * b + g]["out"]
        full[b] = acc + bo
    return full


def kernel(**inputs):
    nc = _get_program()
    in_maps = host_prep(inputs)
    res = run_bass_kernel_spmd(nc, in_maps, list(range(8)))
    return _gather(res.results, np.asarray(inputs["bo"], np.float32))


def kernel_traced(**inputs):
    """Like kernel() but with NTFF tracing; returns (output, BassKernelResults)."""
    nc = _get_program()
    in_maps = host_prep(inputs)
    res = run_bass_kernel_spmd(nc, in_maps, list(range(8)), trace=True)
    return _gather(res.results, np.asarray(inputs["bo"], np.float32)), res
